# revision 11
# baseline (speedup 1.0000x reference)
"""Trainium (trn2) kernel for CurvedRoIExtractor (nn_CurvedRoIExtractor_28295244546862).

kernel(**inputs) takes the FULL inputs (as produced by setup_inputs()) and
returns the FULL output [2, 256, 256, 3, 16] f32.

Sharding: 8 cores = (batch b in {0,1}) x (64-roi quarter).  The core's
feature maps are pre-transposed on the host to a channel-last fp16 table.
Levels 0-2 are stored VERTICALLY PAIR-INTERLEAVED (table row 2r = pixel
row r, row 2r+1 = pixel row r+W), so the four bilinear neighbors of a
sample point occupy four consecutive table rows starting at 2*(y0*W+x0):
[(x0,y0), (x0,y1), (x1,y0), (x1,y1)].  One dma_gather descriptor
(elem_size=1024 fp16 = 2 KB, elem_step=512: windows overlap on the
doubled row grid) fetches a point's whole quad — one token per point,
half the SWDGE descriptor-generation work of a pair layout.  The
bilinear weighted sum runs on TensorE: per (128-point block, level,
neighbor n) one matmul with diagonal lhsT[q, j] = (q==j) * w_n[j]
(host-precomputed weights; diagonals built on DVE as ident x broadcast).
Level 3 (20x20) is computed DENSELY: the whole level-3 table (512 rows
with pad) sits in SBUF and each block adds 4 matmuls with a
host-precomputed sparse-in-dense weight matrix W3[pixel, point].  All 16
matmuls per block accumulate in PSUM; PSUM (f32) is staged to fp16 on
the Scalar engine into [128, 512] tiles (1 KB DMA packets) and written
out tile-major; the host reorders + upcasts.

The big prep tables (idx, weights) load via plain HWDGE DMAs split so
chunk0's slice lands first; W3/T3 load via dma_gather during the
pipeline-fill window.
"""

from contextlib import ExitStack

import numpy as np

import concourse.bass as bass
import concourse.mybir as mybir
import concourse.tile as tile
from concourse import library_config
from concourse.bass_utils import run_bass_kernel_spmd
from concourse.tile import add_dep_helper

F32 = mybir.dt.float32
F16 = mybir.dt.float16
I16 = mybir.dt.int16
AOP = mybir.AluOpType

# (W, H, interleaved base row) of each gathered level
LEVELS = [
    (160, 160, 0),
    (80, 80, 51200),
    (40, 40, 64000),
]
L3_BASE = 67200       # dense level-3 table: 512 rows (400 + pad)
ROWS = 67712
NGLVL = 3
C = 256               # channels
BS = 2
NROI_TOTAL = 256
WP = 16
OUT_H = 3
NPTS = 3072           # per core: 64 rois * 3 * 16
NCHUNK = 6            # chunks of 512 points
# per chunk: one 512-idx L0 gather + one 1024-idx merged L1+L2 gather
# (L1/L2 anchors offset by 0 / 6400 within the L1q..L2q table region)
ICOLS = NCHUNK * 96 + 32 + 8  # chunk idx + t3 idx (512) + w3 idx (128)
NOUT = NPTS // 256    # 12 output tiles of [128, 512]
NBLK = NPTS // 128    # 24 128-point blocks
W3COLS = NBLK * 4 * 128  # 12288
WMW = 128 + NCHUNK * 48  # ident + per-chunk weight cols


def _fix_waits(nc, max_waits=1):
    """The walrus build in this env rejects >1 sem wait per instruction;
    spill extras onto preceding NOPs on the same engine."""
    for func in nc.m.functions:
        for bb in func.blocks:
            insts = bb.instructions
            for ins in list(insts):
                si = ins.sync_info
                if si is None:
                    continue
                w = list(si.on_wait)
                if len(w) > max_waits:
                    si.on_wait = w[:max_waits]
                    pos = insts.index(ins)
                    extra = w[max_waits:]
                    for k in range(0, len(extra), max_waits):
                        nop = mybir.InstNoOp(
                            name=f"{ins.name}-wf{k}",
                            engine=ins.engine,
                            bass_nofuse=True,
                            sync_info=mybir.SyncInfo(
                                on_wait=extra[k : k + max_waits], on_update=[]
                            ),
                        )
                        insts.insert(pos, nop)
                        pos += 1


def _build_kernel(fix=True):
    """Per-core program.  See module docstring for the layout."""
    nc = bass.Bass("TRN2", target_bir_lowering=False, num_devices=8,
                   num_swdge_queues=4)
    tf = nc.dram_tensor("tfeats", [ROWS, C], F16, kind="ExternalInput")
    idxd = nc.dram_tensor("idx", [128, ICOLS], I16, kind="ExternalInput")
    wmd = nc.dram_tensor("wm", [128, WMW], F16, kind="ExternalInput")
    w3d = nc.dram_tensor("w3", [128, W3COLS], F16, kind="ExternalInput")
    outd = nc.dram_tensor("out", [NOUT, 128, 2 * C], F16,
                          kind="ExternalOutput")
    tf_h = tf[:].tensor

    with tile.TileContext(nc) as tc, ExitStack() as ctx:
        prep = ctx.enter_context(tc.tile_pool(name="prep", bufs=1))
        gpool = ctx.enter_context(tc.tile_pool(name="g", bufs=3))
        lpool = ctx.enter_context(tc.tile_pool(name="lt", bufs=2))
        opool = ctx.enter_context(tc.tile_pool(name="o", bufs=1))
        ppool = ctx.enter_context(tc.tile_pool(name="ps", bufs=6, space="PSUM"))

        nc.gpsimd.load_library(library_config.attnmlp)

        idxt = prep.tile([128, ICOLS], I16, tag="idx")
        wmt = prep.tile([128, WMW], F16, tag="wm")
        w3t = prep.tile([128, 1, W3COLS], F16, tag="w3")
        t3 = prep.tile([128, 4, C], F16, tag="t3")
        # chunk0-level0 idx first so its gather can launch ASAP
        nc.sync.dma_start(idxt[:, 0:32], idxd[:, 0:32])
        nc.sync.dma_start(idxt[:, 32:ICOLS], idxd[:, 32:ICOLS])
        nc.sync.dma_start(wmt[:], wmd[:])
        ident = wmt[:, 0:128]

        reg128 = nc.gpsimd.to_reg(128)
        reg512 = nc.gpsimd.to_reg(512)

        # per-chunk diagonal lhsT stacks, built on DVE:
        # lt[q, l*16 + b*4 + n, j] = ident[q, j] * w[q, chunk-col]
        def build_lt(ch):
            lt = lpool.tile([128, NGLVL * 16, 128], F16, tag="lt")
            nc.vector.tensor_tensor(
                lt[:],
                ident.unsqueeze(1).to_broadcast([128, NGLVL * 16, 128]),
                wmt[:, 128 + ch * 48 : 128 + (ch + 1) * 48]
                    .unsqueeze(2).to_broadcast([128, NGLVL * 16, 128]),
                AOP.mult,
            )
            return lt

        reg1024 = nc.gpsimd.to_reg(1024)

        prev_mm = None
        first_loads_issued = False
        for ch in range(NCHUNK):
            W0, H0, base0 = LEVELS[0]
            gt0 = gpool.tile([128, 4, 4 * C], F16, tag="g0")
            nc.gpsimd.dma_gather(
                out_ap=gt0[:],
                in_ap=bass.AP(tf_h, base0 * C,
                              [[2 * C, W0 * H0 - 1], [1, 4 * C]]),
                idxs_ap=idxt[:, ch * 96 : ch * 96 + 32],
                num_idxs=512,
                num_idxs_reg=reg512,
                elem_size=4 * C,
                elem_step=2 * C,
                queue_num=(2 * ch) % 4,
            )
            # merged L1+L2: tokens 0-511 level 1, 512-1023 level 2
            base12 = LEVELS[1][2]
            slots12 = (ROWS - 512 - base12) // 2  # 8000 anchor slots
            gt12 = gpool.tile([128, 8, 4 * C], F16, tag="g12")
            nc.gpsimd.dma_gather(
                out_ap=gt12[:],
                in_ap=bass.AP(tf_h, base12 * C,
                              [[2 * C, slots12 - 1], [1, 4 * C]]),
                idxs_ap=idxt[:, ch * 96 + 32 : ch * 96 + 96],
                num_idxs=1024,
                num_idxs_reg=reg1024,
                elem_size=4 * C,
                elem_step=2 * C,
                queue_num=(2 * ch + 1) % 4,
            )
            gts = [gt0, gt12]
            if not first_loads_issued:
                # level-3 table + dense weights, loaded once via gathers
                first_loads_issued = True
                nc.gpsimd.dma_gather(
                    out_ap=t3[:],
                    in_ap=bass.AP(tf_h, L3_BASE * C, [[C, 512], [1, C]]),
                    idxs_ap=idxt[:, NCHUNK * 96 : NCHUNK * 96 + 32],
                    num_idxs=512,
                    num_idxs_reg=reg512,
                    elem_size=C,
                    queue_num=2,
                )
                nc.gpsimd.dma_gather(
                    out_ap=w3t[:],
                    in_ap=w3d[:],
                    idxs_ap=idxt[:, NCHUNK * 96 + 32 : NCHUNK * 96 + 40],
                    num_idxs=128,
                    num_idxs_reg=reg128,
                    elem_size=W3COLS,
                    queue_num=3,
                )
            lt = build_lt(ch)
            for tpair in range(2):     # output tile = 2 blocks = 256 pts
                so = opool.tile([128, 2 * C], F16, tag=f"so{ch * 2 + tpair}")
                for h in range(2):
                    b = tpair * 2 + h
                    blkg = ch * 4 + b
                    ps = ppool.tile([128, C], F32, tag="ps")
                    k = 0
                    for l in range(NGLVL):
                        rhs_t = gts[0] if l == 0 else gts[1]
                        rhs_b = b if l < 2 else 4 + b
                        for n in range(4):
                            mm = nc.tensor.matmul(
                                ps[:],
                                lt[:, l * 16 + b * 4 + n, :],
                                rhs_t[:, rhs_b, n * C : (n + 1) * C],
                                start=(k == 0),
                                stop=False,
                            )
                            # accumulation chains sharing a PSUM bank must
                            # not interleave -> force PE program order
                            if prev_mm is not None:
                                add_dep_helper(mm.ins, prev_mm.ins,
                                               sync=False)
                            prev_mm = mm
                            k += 1
                    for kt in range(4):   # dense level-3
                        off = (blkg * 4 + kt) * 128
                        mm = nc.tensor.matmul(
                            ps[:],
                            w3t[:, 0, off : off + 128],
                            t3[:, kt, :],
                            start=False,
                            stop=(kt == 3),
                        )
                        add_dep_helper(mm.ins, prev_mm.ins, sync=False)
                        prev_mm = mm
                    nc.scalar.activation(so[:, h * C : (h + 1) * C], ps[:],
                                         mybir.ActivationFunctionType.Copy)
                nc.sync.dma_start(outd[ch * 2 + tpair], so[:])

    mybir.codegen_inst_isa_subclasses(nc)
    if fix:
        _fix_waits(nc)
    return nc


# ---------------------------------------------------------------------------
# Host-side prep

def _wrap128(flat):
    """Token-order idx list -> wrapped [16, n/16] replicated to [128, ...]."""
    w = flat.reshape(-1, 16).T.astype(np.int16)
    return np.tile(w, (8, 1))


def _host_prep_points(center_b, boundary_b, roi0, nroi):
    """Returns (idx [128, ICOLS] i16, wm [128, WMW] f16, w3 [128, .] f16)."""
    bp = boundary_b[roi0 : roi0 + nroi]      # [nroi, Wp, 4]
    cp = center_b[roi0 : roi0 + nroi]        # [nroi, Wp, 2]
    sp = np.stack([bp[..., 0:2], cp, bp[..., 2:4]], axis=1)  # [nroi,3,Wp,2]
    gx = np.ascontiguousarray(sp[..., 0].transpose(1, 2, 0)).reshape(-1)
    gy = np.ascontiguousarray(sp[..., 1].transpose(1, 2, 0)).reshape(-1)
    gx = gx.astype(np.float32)
    gy = gy.astype(np.float32)

    idx = np.zeros((128, ICOLS), np.int16)
    wm = np.zeros((128, WMW), np.float16)
    wm[:, 0:128] = np.eye(128, dtype=np.float16)

    def lvl_geom(W, H):
        x = ((gx + np.float32(1.0)) * np.float32(0.5)) * np.float32(W - 1)
        y = ((gy + np.float32(1.0)) * np.float32(0.5)) * np.float32(H - 1)
        x0 = np.floor(x)
        y0 = np.floor(y)
        return x0.astype(np.int32), y0.astype(np.int32), x - x0, y - y0

    anchors = []
    for l in range(NGLVL):
        W, H, base = LEVELS[l]
        x0, y0, wx, wy = lvl_geom(W, H)
        anchors.append(y0 * W + x0)
        # quad order: (x0,y0), (x0,y1), (x1,y0), (x1,y1)
        wq = np.stack([(1 - wx) * (1 - wy), (1 - wx) * wy,
                       wx * (1 - wy), wx * wy])   # [4, NPTS]
        for ch in range(NCHUNK):
            for b in range(4):
                p128 = ch * 512 + b * 128 + np.arange(128)
                for n in range(4):
                    wm[:, 128 + ch * 48 + l * 16 + b * 4 + n] = \
                        wq[n, p128].astype(np.float16)

    # L1 anchor slots are level-local; L2 slots offset by L1's slot count
    off2 = (LEVELS[2][2] - LEVELS[1][2]) // 2  # 6400
    for ch in range(NCHUNK):
        sl = slice(ch * 512, (ch + 1) * 512)
        idx[:, ch * 96 : ch * 96 + 32] = _wrap128(anchors[0][sl])
        idx[:, ch * 96 + 32 : ch * 96 + 96] = _wrap128(
            np.concatenate([anchors[1][sl], anchors[2][sl] + off2]))

    # t3 / w3 bootstrap idx
    idx[:, NCHUNK * 96 : NCHUNK * 96 + 32] = _wrap128(np.arange(512))
    idx[:, NCHUNK * 96 + 32 : NCHUNK * 96 + 40] = _wrap128(np.arange(128))

    # dense level-3 weights: W3[pix, pt] (512 pix rows with pad, 3072 pts)
    W, H = 20, 20
    x0, y0, wx, wy = lvl_geom(W, H)
    w3full = np.zeros((512, NPTS), np.float32)
    pts = np.arange(NPTS)
    for dy, dx, wgt in ((0, 0, (1 - wx) * (1 - wy)), (0, 1, wx * (1 - wy)),
                        (1, 0, (1 - wx) * wy), (1, 1, wx * wy)):
        w3full[(y0 + dy) * W + (x0 + dx), pts] = wgt
    # w3[p, (blk*4 + k)*128 + j] = w3full[k*128 + p, blk*128 + j]
    w3 = np.ascontiguousarray(
        w3full.reshape(4, 128, NBLK, 128)     # [k, p, blk, j]
        .transpose(1, 2, 0, 3)                # [p, blk, k, j]
        .reshape(128, W3COLS)).astype(np.float16)
    return idx, wm, w3


def _host_tfeats(feats_b_list):
    """Channel-last fp16 table: levels 0-2 vertically pair-interleaved,
    then the level-3 tile (400 rows + pad to 512)."""
    parts = []
    for f in feats_b_list[:NGLVL]:
        Cc, H, W = f.shape
        lvl = np.ascontiguousarray(f.reshape(Cc, -1).T).astype(np.float16)
        q = np.zeros((2 * H * W, Cc), np.float16)
        q[0::2] = lvl
        q[1::2][: W * (H - 1)] = lvl[W:]
        parts.append(q)
    f3 = feats_b_list[3]
    lvl3 = np.ascontiguousarray(
        f3.reshape(f3.shape[0], -1).T).astype(np.float16)
    t3 = np.zeros((512, C), np.float16)
    t3[:400] = lvl3
    parts.append(t3)
    tfx = np.concatenate(parts, axis=0)
    assert tfx.shape[0] == ROWS, tfx.shape
    return np.ascontiguousarray(tfx)


_CACHE = {}


def _get_nc():
    if "nc" not in _CACHE:
        _CACHE["nc"] = _build_kernel()
    return _CACHE["nc"]


def kernel(feats0, feats1, feats2, feats3, center_points, boundary_points,
           _want_trace=False, _trace_dir=None):
    feats0 = np.asarray(feats0, dtype=np.float32)
    feats1 = np.asarray(feats1, dtype=np.float32)
    feats2 = np.asarray(feats2, dtype=np.float32)
    feats3 = np.asarray(feats3, dtype=np.float32)
    center_points = np.asarray(center_points, dtype=np.float32)
    boundary_points = np.asarray(boundary_points, dtype=np.float32)

    nc = _get_nc()
    tfeats = [
        _host_tfeats([feats0[b], feats1[b], feats2[b], feats3[b]])
        for b in range(BS)
    ]
    nroi = NROI_TOTAL // 4  # 64 rois per core
    in_maps = []
    for core in range(8):
        b = core // 4
        roi0 = (core % 4) * nroi
        idx, wm, w3 = _host_prep_points(
            center_points[b], boundary_points[b], roi0, nroi)
        in_maps.append(
            {"tfeats": tfeats[b], "idx": idx, "wm": wm, "w3": w3})

    kwargs = {}
    if _want_trace:
        kwargs = {"trace": True}
        if _trace_dir is not None:
            kwargs["tmpdir"] = _trace_dir
    res = run_bass_kernel_spmd(nc, in_maps, core_ids=list(range(8)), **kwargs)

    out = np.empty((BS, NROI_TOTAL, C, OUT_H, WP), np.float32)
    for core in range(8):
        b = core // 4
        roi0 = (core % 4) * nroi
        dev = res.results[core]["out"]          # [12, 128, 512] f16
        pts = (dev.astype(np.float32)
               .reshape(NOUT, 128, 2, C)
               .transpose(0, 2, 1, 3)
               .reshape(NPTS, C))               # rows (h, w, roi')
        o = pts.reshape(OUT_H, WP, nroi, C)
        out[b, roi0 : roi0 + nroi] = o.transpose(2, 3, 0, 1)
    if _want_trace:
        return out, res
    return out


# revision 15
# speedup vs baseline: 1.0589x; 1.0589x over previous
"""Trainium (trn2) kernel for CurvedRoIExtractor (nn_CurvedRoIExtractor_28295244546862).

kernel(**inputs) takes the FULL inputs (as produced by setup_inputs()) and
returns the FULL output [2, 256, 256, 3, 16] f32.

Sharding: 8 cores = (batch b in {0,1}) x (64-roi quarter).  The core's
feature maps are pre-transposed on the host to a channel-last fp16 table.
Levels 0-2 are stored VERTICALLY PAIR-INTERLEAVED (table row 2r = pixel
row r, row 2r+1 = pixel row r+W), so the four bilinear neighbors of a
sample point occupy four consecutive table rows starting at 2*(y0*W+x0):
[(x0,y0), (x0,y1), (x1,y0), (x1,y1)].  One dma_gather descriptor
(elem_size=1024 fp16 = 2 KB, elem_step=512: windows overlap on the
doubled row grid) fetches a point's whole quad — one token per point,
half the SWDGE descriptor-generation work of a pair layout.  The
bilinear weighted sum runs on TensorE: per (128-point block, level,
neighbor n) one matmul with diagonal lhsT[q, j] = (q==j) * w_n[j]
(host-precomputed weights; diagonals built on DVE as ident x broadcast).
Level 3 (20x20) is computed DENSELY: the whole level-3 table (512 rows
with pad) sits in SBUF and each block adds 4 matmuls with a
host-precomputed sparse-in-dense weight matrix W3[pixel, point].  All 16
matmuls per block accumulate in PSUM; PSUM (f32) is staged to fp16 on
the Scalar engine into [128, 512] tiles (1 KB DMA packets) and written
out tile-major; the host reorders + upcasts.

The big prep tables (idx, weights) load via plain HWDGE DMAs split so
chunk0's slice lands first; W3/T3 load via dma_gather during the
pipeline-fill window.
"""

from contextlib import ExitStack

import numpy as np

import concourse.bass as bass
import concourse.mybir as mybir
import concourse.tile as tile
from concourse import library_config
from concourse.bass_utils import run_bass_kernel_spmd
from concourse.tile import add_dep_helper

F32 = mybir.dt.float32
F16 = mybir.dt.float16
I16 = mybir.dt.int16
AOP = mybir.AluOpType

# (W, H, interleaved base row) of each gathered level
LEVELS = [
    (160, 160, 0),
    (80, 80, 51200),
    (40, 40, 64000),
]
L3_BASE = 67200       # dense level-3 table: 512 rows (400 + pad)
ROWS = 67712
NGLVL = 3
C = 256               # channels
BS = 2
NROI_TOTAL = 256
WP = 16
OUT_H = 3
NPTS = 3072           # per core: 64 rois * 3 * 16
NCHUNK = 6            # chunks of 512 points
# per chunk: one 512-idx L0 gather + one 1024-idx merged L1+L2 gather
# (L1/L2 anchors offset by 0 / 6400 within the L1q..L2q table region)
ICOLS = NCHUNK * 96 + 32 + 8  # chunk idx + t3 idx (512) + w3 idx (128)
NOUT = NPTS // 256    # 12 output tiles of [128, 512]
NBLK = NPTS // 128    # 24 128-point blocks
W3COLS = NBLK * 4 * 128  # 12288
WMW = 128 + NCHUNK * 48  # ident + per-chunk weight cols


def _fix_waits(nc, max_waits=1):
    """The walrus build in this env rejects >1 sem wait per instruction;
    spill extras onto preceding NOPs on the same engine."""
    for func in nc.m.functions:
        for bb in func.blocks:
            insts = bb.instructions
            for ins in list(insts):
                si = ins.sync_info
                if si is None:
                    continue
                w = list(si.on_wait)
                if len(w) > max_waits:
                    si.on_wait = w[:max_waits]
                    pos = insts.index(ins)
                    extra = w[max_waits:]
                    for k in range(0, len(extra), max_waits):
                        nop = mybir.InstNoOp(
                            name=f"{ins.name}-wf{k}",
                            engine=ins.engine,
                            bass_nofuse=True,
                            sync_info=mybir.SyncInfo(
                                on_wait=extra[k : k + max_waits], on_update=[]
                            ),
                        )
                        insts.insert(pos, nop)
                        pos += 1


def _build_kernel(fix=True):
    """Per-core program.  See module docstring for the layout."""
    nc = bass.Bass("TRN2", target_bir_lowering=False, num_devices=8,
                   num_swdge_queues=4)
    tf = nc.dram_tensor("tfeats", [ROWS, C], F16, kind="ExternalInput")
    idxd = nc.dram_tensor("idx", [128, ICOLS], I16, kind="ExternalInput")
    wmd = nc.dram_tensor("wm", [128, WMW], F16, kind="ExternalInput")
    w3d = nc.dram_tensor("w3", [128, W3COLS], F16, kind="ExternalInput")
    outd = nc.dram_tensor("out", [NOUT, 128, 2 * C], F16,
                          kind="ExternalOutput")
    tf_h = tf[:].tensor

    with tile.TileContext(nc) as tc, ExitStack() as ctx:
        prep = ctx.enter_context(tc.tile_pool(name="prep", bufs=1))
        gpool = ctx.enter_context(tc.tile_pool(name="g", bufs=3))
        lpool = ctx.enter_context(tc.tile_pool(name="lt", bufs=3))
        opool = ctx.enter_context(tc.tile_pool(name="o", bufs=1))
        ppool = ctx.enter_context(tc.tile_pool(name="ps", bufs=6, space="PSUM"))

        nc.gpsimd.load_library(library_config.attnmlp)

        idxt = prep.tile([128, ICOLS], I16, tag="idx")
        wmt = prep.tile([128, WMW], F16, tag="wm")
        w3t = prep.tile([128, 1, W3COLS], F16, tag="w3")
        t3 = prep.tile([128, 4, C], F16, tag="t3")
        # chunk0-level0 idx first so its gather can launch ASAP
        nc.sync.dma_start(idxt[:, 0:32], idxd[:, 0:32])
        nc.sync.dma_start(idxt[:, 32:ICOLS], idxd[:, 32:ICOLS])
        nc.sync.dma_start(wmt[:], wmd[:])
        ident = wmt[:, 0:128]

        reg128 = nc.gpsimd.to_reg(128)
        reg512 = nc.gpsimd.to_reg(512)

        # per-chunk diagonal lhsT stacks, built on DVE:
        # lt[q, l*16 + b*4 + n, j] = ident[q, j] * w[q, chunk-col]
        def build_lt(ch):
            lt = lpool.tile([128, NGLVL * 16, 128], F16, tag="lt")
            nc.vector.tensor_tensor(
                lt[:],
                ident.unsqueeze(1).to_broadcast([128, NGLVL * 16, 128]),
                wmt[:, 128 + ch * 48 : 128 + (ch + 1) * 48]
                    .unsqueeze(2).to_broadcast([128, NGLVL * 16, 128]),
                AOP.mult,
            )
            return lt

        reg1024 = nc.gpsimd.to_reg(1024)

        # byte-balanced queue schedule: L0 gathers are 1 MB, L1+L2 2 MB,
        # W3 halves 1.55 MB, T3 0.26 MB -> ~5.5 MB per queue
        q_l0 = [0, 2, 1, 3, 0, 2]
        q_l12 = [1, 3, 2, 0, 3, 1]

        prev_mm = None
        first_loads_issued = False
        for ch in range(NCHUNK):
            W0, H0, base0 = LEVELS[0]
            gt0 = gpool.tile([128, 4, 4 * C], F16, tag="g0")
            nc.gpsimd.dma_gather(
                out_ap=gt0[:],
                in_ap=bass.AP(tf_h, base0 * C,
                              [[2 * C, W0 * H0 - 1], [1, 4 * C]]),
                idxs_ap=idxt[:, ch * 96 : ch * 96 + 32],
                num_idxs=512,
                num_idxs_reg=reg512,
                elem_size=4 * C,
                elem_step=2 * C,
                queue_num=q_l0[ch],
            )
            # merged L1+L2: tokens 0-511 level 1, 512-1023 level 2
            base12 = LEVELS[1][2]
            slots12 = (ROWS - 512 - base12) // 2  # 8000 anchor slots
            gt12 = gpool.tile([128, 8, 4 * C], F16, tag="g12")
            nc.gpsimd.dma_gather(
                out_ap=gt12[:],
                in_ap=bass.AP(tf_h, base12 * C,
                              [[2 * C, slots12 - 1], [1, 4 * C]]),
                idxs_ap=idxt[:, ch * 96 + 32 : ch * 96 + 96],
                num_idxs=1024,
                num_idxs_reg=reg1024,
                elem_size=4 * C,
                elem_step=2 * C,
                queue_num=q_l12[ch],
            )
            gts = [gt0, gt12]
            if not first_loads_issued:
                # level-3 table + dense weights, loaded once via gathers
                first_loads_issued = True
                nc.gpsimd.dma_gather(
                    out_ap=t3[:],
                    in_ap=bass.AP(tf_h, L3_BASE * C, [[C, 512], [1, C]]),
                    idxs_ap=idxt[:, NCHUNK * 96 : NCHUNK * 96 + 32],
                    num_idxs=512,
                    num_idxs_reg=reg512,
                    elem_size=C,
                    queue_num=2,
                )
                w3_h = w3d[:].tensor
                for wh in range(2):
                    half = W3COLS // 2
                    nc.gpsimd.dma_gather(
                        out_ap=w3t[:, :, wh * half : (wh + 1) * half],
                        in_ap=bass.AP(w3_h, wh * half,
                                      [[W3COLS, 128], [1, half]]),
                        idxs_ap=idxt[:, NCHUNK * 96 + 32 : NCHUNK * 96 + 40],
                        num_idxs=128,
                        num_idxs_reg=reg128,
                        elem_size=half,
                        elem_step=W3COLS,
                        queue_num=1 + 2 * wh,
                    )
            lt = build_lt(ch)
            for tpair in range(2):     # output tile = 2 blocks = 256 pts
                so = opool.tile([128, 2 * C], F16, tag=f"so{ch * 2 + tpair}")
                for h in range(2):
                    b = tpair * 2 + h
                    blkg = ch * 4 + b
                    ps = ppool.tile([128, C], F32, tag="ps")
                    k = 0
                    for l in range(NGLVL):
                        rhs_t = gts[0] if l == 0 else gts[1]
                        rhs_b = b if l < 2 else 4 + b
                        for n in range(4):
                            mm = nc.tensor.matmul(
                                ps[:],
                                lt[:, l * 16 + b * 4 + n, :],
                                rhs_t[:, rhs_b, n * C : (n + 1) * C],
                                start=(k == 0),
                                stop=False,
                            )
                            # accumulation chains sharing a PSUM bank must
                            # not interleave -> force PE program order
                            if prev_mm is not None:
                                add_dep_helper(mm.ins, prev_mm.ins,
                                               sync=False)
                            prev_mm = mm
                            k += 1
                    for kt in range(4):   # dense level-3
                        off = (blkg * 4 + kt) * 128
                        mm = nc.tensor.matmul(
                            ps[:],
                            w3t[:, 0, off : off + 128],
                            t3[:, kt, :],
                            start=False,
                            stop=(kt == 3),
                        )
                        add_dep_helper(mm.ins, prev_mm.ins, sync=False)
                        prev_mm = mm
                    nc.scalar.activation(so[:, h * C : (h + 1) * C], ps[:],
                                         mybir.ActivationFunctionType.Copy)
                nc.sync.dma_start(outd[ch * 2 + tpair], so[:])

    mybir.codegen_inst_isa_subclasses(nc)
    if fix:
        _fix_waits(nc)
    return nc


# ---------------------------------------------------------------------------
# Host-side prep

def _wrap128(flat):
    """Token-order idx list -> wrapped [16, n/16] replicated to [128, ...]."""
    w = flat.reshape(-1, 16).T.astype(np.int16)
    return np.tile(w, (8, 1))


def _host_prep_points(center_b, boundary_b, roi0, nroi):
    """Returns (idx [128, ICOLS] i16, wm [128, WMW] f16, w3 [128, .] f16)."""
    bp = boundary_b[roi0 : roi0 + nroi]      # [nroi, Wp, 4]
    cp = center_b[roi0 : roi0 + nroi]        # [nroi, Wp, 2]
    sp = np.stack([bp[..., 0:2], cp, bp[..., 2:4]], axis=1)  # [nroi,3,Wp,2]
    gx = np.ascontiguousarray(sp[..., 0].transpose(1, 2, 0)).reshape(-1)
    gy = np.ascontiguousarray(sp[..., 1].transpose(1, 2, 0)).reshape(-1)
    gx = gx.astype(np.float32)
    gy = gy.astype(np.float32)

    idx = np.zeros((128, ICOLS), np.int16)
    wm = np.zeros((128, WMW), np.float16)
    wm[:, 0:128] = np.eye(128, dtype=np.float16)

    def lvl_geom(W, H):
        x = ((gx + np.float32(1.0)) * np.float32(0.5)) * np.float32(W - 1)
        y = ((gy + np.float32(1.0)) * np.float32(0.5)) * np.float32(H - 1)
        x0 = np.floor(x)
        y0 = np.floor(y)
        return x0.astype(np.int32), y0.astype(np.int32), x - x0, y - y0

    anchors = []
    for l in range(NGLVL):
        W, H, base = LEVELS[l]
        x0, y0, wx, wy = lvl_geom(W, H)
        anchors.append(y0 * W + x0)
        # quad order: (x0,y0), (x0,y1), (x1,y0), (x1,y1)
        wq = np.stack([(1 - wx) * (1 - wy), (1 - wx) * wy,
                       wx * (1 - wy), wx * wy])   # [4, NPTS]
        for ch in range(NCHUNK):
            for b in range(4):
                p128 = ch * 512 + b * 128 + np.arange(128)
                for n in range(4):
                    wm[:, 128 + ch * 48 + l * 16 + b * 4 + n] = \
                        wq[n, p128].astype(np.float16)

    # L1 anchor slots are level-local; L2 slots offset by L1's slot count
    off2 = (LEVELS[2][2] - LEVELS[1][2]) // 2  # 6400
    for ch in range(NCHUNK):
        sl = slice(ch * 512, (ch + 1) * 512)
        idx[:, ch * 96 : ch * 96 + 32] = _wrap128(anchors[0][sl])
        idx[:, ch * 96 + 32 : ch * 96 + 96] = _wrap128(
            np.concatenate([anchors[1][sl], anchors[2][sl] + off2]))

    # t3 / w3 bootstrap idx
    idx[:, NCHUNK * 96 : NCHUNK * 96 + 32] = _wrap128(np.arange(512))
    idx[:, NCHUNK * 96 + 32 : NCHUNK * 96 + 40] = _wrap128(np.arange(128))

    # dense level-3 weights: W3[pix, pt] (512 pix rows with pad, 3072 pts)
    W, H = 20, 20
    x0, y0, wx, wy = lvl_geom(W, H)
    w3full = np.zeros((512, NPTS), np.float32)
    pts = np.arange(NPTS)
    for dy, dx, wgt in ((0, 0, (1 - wx) * (1 - wy)), (0, 1, wx * (1 - wy)),
                        (1, 0, (1 - wx) * wy), (1, 1, wx * wy)):
        w3full[(y0 + dy) * W + (x0 + dx), pts] = wgt
    # w3[p, (blk*4 + k)*128 + j] = w3full[k*128 + p, blk*128 + j]
    w3 = np.ascontiguousarray(
        w3full.reshape(4, 128, NBLK, 128)     # [k, p, blk, j]
        .transpose(1, 2, 0, 3)                # [p, blk, k, j]
        .reshape(128, W3COLS)).astype(np.float16)
    return idx, wm, w3


def _host_tfeats(feats_b_list):
    """Channel-last fp16 table: levels 0-2 vertically pair-interleaved,
    then the level-3 tile (400 rows + pad to 512)."""
    parts = []
    for f in feats_b_list[:NGLVL]:
        Cc, H, W = f.shape
        lvl = np.ascontiguousarray(f.reshape(Cc, -1).T).astype(np.float16)
        q = np.zeros((2 * H * W, Cc), np.float16)
        q[0::2] = lvl
        q[1::2][: W * (H - 1)] = lvl[W:]
        parts.append(q)
    f3 = feats_b_list[3]
    lvl3 = np.ascontiguousarray(
        f3.reshape(f3.shape[0], -1).T).astype(np.float16)
    t3 = np.zeros((512, C), np.float16)
    t3[:400] = lvl3
    parts.append(t3)
    tfx = np.concatenate(parts, axis=0)
    assert tfx.shape[0] == ROWS, tfx.shape
    return np.ascontiguousarray(tfx)


_CACHE = {}


def _get_nc():
    if "nc" not in _CACHE:
        _CACHE["nc"] = _build_kernel()
    return _CACHE["nc"]


def kernel(feats0, feats1, feats2, feats3, center_points, boundary_points,
           _want_trace=False, _trace_dir=None):
    feats0 = np.asarray(feats0, dtype=np.float32)
    feats1 = np.asarray(feats1, dtype=np.float32)
    feats2 = np.asarray(feats2, dtype=np.float32)
    feats3 = np.asarray(feats3, dtype=np.float32)
    center_points = np.asarray(center_points, dtype=np.float32)
    boundary_points = np.asarray(boundary_points, dtype=np.float32)

    nc = _get_nc()
    tfeats = [
        _host_tfeats([feats0[b], feats1[b], feats2[b], feats3[b]])
        for b in range(BS)
    ]
    nroi = NROI_TOTAL // 4  # 64 rois per core
    in_maps = []
    for core in range(8):
        b = core // 4
        roi0 = (core % 4) * nroi
        idx, wm, w3 = _host_prep_points(
            center_points[b], boundary_points[b], roi0, nroi)
        in_maps.append(
            {"tfeats": tfeats[b], "idx": idx, "wm": wm, "w3": w3})

    kwargs = {}
    if _want_trace:
        kwargs = {"trace": True}
        if _trace_dir is not None:
            kwargs["tmpdir"] = _trace_dir
    res = run_bass_kernel_spmd(nc, in_maps, core_ids=list(range(8)), **kwargs)

    out = np.empty((BS, NROI_TOTAL, C, OUT_H, WP), np.float32)
    for core in range(8):
        b = core // 4
        roi0 = (core % 4) * nroi
        dev = res.results[core]["out"]          # [12, 128, 512] f16
        pts = (dev.astype(np.float32)
               .reshape(NOUT, 128, 2, C)
               .transpose(0, 2, 1, 3)
               .reshape(NPTS, C))               # rows (h, w, roi')
        o = pts.reshape(OUT_H, WP, nroi, C)
        out[b, roi0 : roi0 + nroi] = o.transpose(2, 3, 0, 1)
    if _want_trace:
        return out, res
    return out
